# revision 19
# baseline (speedup 1.0000x reference)
"""Trainium2 Bass kernel for nn_DMAQer_DVD (DMAQ-style mixer + per-row GAT).

Layout strategy (pure data parallel over 8 cores, 8192 rows/core):
  - Host pre-transposes states / hidden_states so the device never transposes.
  - Per 128-row tile, all matmuls run on PE in natural layouts:
      * hyper MLP:  h1T = relu(W1a.T @ S_T)  ->  wf = |h1a.T @ W2a|   (bias via ones-rows)
      * GAT:        hp[(b,a), (h,d)] = H.T-chunk.T @ Wg  per 16-row chunk
                    e_i/e_j via folded attention vectors (Wae)
      * attention applied via block-diagonalised exp(lrelu(e)) as PE stationary,
        with a ones-column producing the softmax denominator for free.
  - Softmax/ELU chains on ACT (exp) + DVE (everything else).

Math simplification used: adv_q = w_final*(q - mq); the V-MLP cancels exactly.
"""

import os
import sys
import numpy as np

sys.path.insert(0, "/opt/trn_rl_repo")

N_CORES = 8
B_TOTAL = 256 * 256          # bs * T
B = B_TOTAL // N_CORES       # 8192 rows per core
N_TILES = B // 128           # 64 tiles of 128 rows
N_AGENTS = 8
STATE_DIM = 168
RNN_DIM = 64
GAT_DIM = 32
N_HEADS = 4
HYPER = 64

_CACHE = {}


def _build_bass(n_tiles=N_TILES, debug=False):
    import concourse.bass as bass
    import concourse.tile as tile
    from concourse import bacc, mybir

    f32 = mybir.dt.float32
    nc = bacc.Bacc("TRN2", target_bir_lowering=False, debug=False)

    # ---- DRAM parameters (per-core shard; same program on all cores) ----
    st = nc.declare_dram_parameter("st", [STATE_DIM + 1, B], f32, isOutput=False)
    ht = nc.declare_dram_parameter("ht", [RNN_DIM, B * N_AGENTS], f32, isOutput=False)
    qm = nc.declare_dram_parameter("qm", [B, 16], f32, isOutput=False)
    w1a = nc.declare_dram_parameter("w1a", [STATE_DIM + 1, HYPER], f32, isOutput=False)
    w2a = nc.declare_dram_parameter("w2a", [HYPER + 1, N_AGENTS], f32, isOutput=False)
    wg = nc.declare_dram_parameter("wg", [RNN_DIM, N_HEADS * GAT_DIM], f32, isOutput=False)
    wae = nc.declare_dram_parameter("wae", [RNN_DIM, 8], f32, isOutput=False)
    wda = nc.declare_dram_parameter("wda", [STATE_DIM + 1, N_HEADS * GAT_DIM], f32, isOutput=False)
    bo = nc.declare_dram_parameter("bo", [128, 128], f32, isOutput=False)
    mc8 = nc.declare_dram_parameter("mc8", [128, 8], f32, isOutput=False)
    mc16 = nc.declare_dram_parameter("mc16", [128, 16], f32, isOutput=False)
    mc16b = nc.declare_dram_parameter("mc16b", [128, 16], f32, isOutput=False)
    ident = nc.declare_dram_parameter("ident", [128, 128], f32, isOutput=False)
    sel3 = nc.declare_dram_parameter("sel3", [8, 128], f32, isOutput=False)
    selk = nc.declare_dram_parameter("selk", [128, 8 * 128], f32, isOutput=False)
    out = nc.declare_dram_parameter("out", [B, 1], f32, isOutput=True)
    dbg = {}
    if debug:
        for nm, shp in [("d_advq", [128, 8]), ("d_aw1", [128, 8]),
                        ("d_awn", [128, 8]), ("d_eall", [128, 256]),
                        ("d_hps", [128, 8 * 4 * 33]), ("d_z", [128, 32]),
                        ("d_g", [128, 1024]), ("d_sp", [128, 1024])]:
            dbg[nm] = nc.declare_dram_parameter(nm, shp, f32, isOutput=True)

    AL = mybir.AluOpType
    AF = mybir.ActivationFunctionType
    AX = mybir.AxisListType

    with tile.TileContext(nc) as tc:
        with (
            tc.tile_pool(name="singles", bufs=1) as singles,
            tc.tile_pool(name="loads", bufs=3) as loads,
            tc.tile_pool(name="work", bufs=2) as work,
            tc.tile_pool(name="bigwork", bufs=2) as bigwork,
            tc.tile_pool(name="ps_small", bufs=2, space="PSUM") as ps_small,
            tc.tile_pool(name="ps_hp", bufs=1, space="PSUM") as ps_hp,
            tc.tile_pool(name="ps_e", bufs=1, space="PSUM") as ps_e,
            tc.tile_pool(name="ps_s", bufs=1, space="PSUM") as ps_s,
        ):
            # ---- constants into SBUF once ----
            w1a_hi = singles.tile([128, HYPER], f32)
            nc.sync.dma_start(w1a_hi[:], w1a[0:128, :])
            w1a_lo = singles.tile([STATE_DIM + 1 - 128, HYPER], f32)
            nc.sync.dma_start(w1a_lo[:], w1a[128:STATE_DIM + 1, :])
            w2a_s = singles.tile([HYPER + 1, N_AGENTS], f32)
            nc.sync.dma_start(w2a_s[:], w2a[:])
            wg_s = singles.tile([RNN_DIM, 128], f32)
            nc.sync.dma_start(wg_s[:], wg[:])
            wae_s = singles.tile([RNN_DIM, 8], f32)
            nc.sync.dma_start(wae_s[:], wae[:])
            wda_hi = singles.tile([128, 128], f32)
            nc.sync.dma_start(wda_hi[:], wda[0:128, :])
            wda_lo = singles.tile([STATE_DIM + 1 - 128, 128], f32)
            nc.sync.dma_start(wda_lo[:], wda[128:STATE_DIM + 1, :])
            bo_s = singles.tile([128, 128], f32)
            nc.sync.dma_start(bo_s[:], bo[:])
            mc8_s = singles.tile([128, 8], f32)
            nc.sync.dma_start(mc8_s[:], mc8[:])
            mc16_s = singles.tile([128, 16], f32)
            nc.sync.dma_start(mc16_s[:], mc16[:])
            mc16b_s = singles.tile([128, 16], f32)
            nc.sync.dma_start(mc16b_s[:], mc16b[:])
            ident_s = singles.tile([128, 128], f32)
            nc.sync.dma_start(ident_s[:], ident[:])
            sel3_s = singles.tile([8, 128], f32)
            nc.sync.dma_start(sel3_s[:], sel3[:])
            selk_s = singles.tile([128, 8, 128], f32)
            nc.sync.dma_start(selk_s[:].rearrange("p k m -> p (k m)"), selk[:])

            for t in range(n_tiles):
                c0 = t * 128

                # ------------- loads -------------
                st_hi = loads.tile([128, 128], f32, tag="st_hi")
                nc.sync.dma_start(st_hi[:], st[0:128, c0:c0 + 128])
                st_lo = loads.tile([STATE_DIM + 1 - 128, 128], f32, tag="st_lo")
                nc.sync.dma_start(st_lo[:], st[128:STATE_DIM + 1, c0:c0 + 128])
                ht_t = loads.tile([RNN_DIM, 1024], f32, tag="ht")
                nc.sync.dma_start(ht_t[:], ht[:, c0 * 8:(c0 + 128) * 8])
                qm_t = loads.tile([128, 16], f32, tag="qm")
                nc.sync.dma_start(qm_t[:], qm[c0:c0 + 128, :])

                # ------------- hyper-MLP track (natural) -------------
                h1T = ps_small.tile([HYPER, 128], f32, tag="psm")
                nc.tensor.matmul(h1T[:], w1a_hi[:], st_hi[:], start=True, stop=False)
                nc.tensor.matmul(h1T[:], w1a_lo[:], st_lo[:], start=False, stop=True)
                h1a = work.tile([HYPER + 1, 128], f32, tag="h1a")
                nc.vector.tensor_scalar(h1a[0:HYPER, :], h1T[:], 0.0, None, AL.max)
                nc.gpsimd.memset(h1a[HYPER:HYPER + 1, :], 1.0)

                wfP = ps_small.tile([128, N_AGENTS], f32, tag="psm")
                nc.tensor.matmul(wfP[:], h1a[:], w2a_s[:], start=True, stop=True)
                advq = work.tile([128, N_AGENTS], f32, tag="advq")
                # advq = |wf| * (q - mq)
                wfs = work.tile([128, N_AGENTS], f32, tag="wfs")
                nc.vector.tensor_copy(wfs[:], wfP[:])
                wfa = work.tile([128, N_AGENTS], f32, tag="wfa")
                nc.vector.scalar_tensor_tensor(wfa[:], wfs[:], -1.0, wfs[:], AL.mult, AL.max)
                dq = work.tile([128, N_AGENTS], f32, tag="dq")
                nc.vector.tensor_sub(dq[:], qm_t[:, 0:8], qm_t[:, 8:16])
                nc.vector.tensor_mul(advq[:], wfa[:], dq[:])

                # ------------- GAT track, (b,a)-partition chunks -------------
                # hp per chunk -> evac into hps [128, k, h, 33] (col 32 = ones)
                hps = bigwork.tile([128, 8, 4, 33], f32, tag="hps")
                eP = ps_e.tile([128, 8, 8], f32, tag="eP")
                for k in range(8):
                    hpP = ps_hp.tile([128, 128], f32, tag="hp")
                    htk = ht_t[:, k * 128:(k + 1) * 128]
                    nc.tensor.matmul(hpP[:], htk, wg_s[:], start=True, stop=True)
                    nc.vector.tensor_copy(
                        hps[:, k, :, 0:32],
                        hpP[:].rearrange("p (h d) -> p h d", h=4),
                    )
                    # e_i -> eP[:, k, 0:4],  e_j -> eP[:, k, 4:8]
                    nc.tensor.matmul(eP[:, k, 0:4], htk, wae_s[:, 0:4], start=True, stop=True)
                    nc.tensor.matmul(eP[:, k, 4:8], htk, wae_s[:, 4:8], start=True, stop=True)
                nc.gpsimd.memset(hps[:, :, :, 32:33], 1.0)

                # eimask[(b,i'),(k,h,i)] = ei * delta_{i,i'}
                em = work.tile([128, 8, 4, 8], f32, tag="em")
                ei_b = bass.AP(
                    tensor=eP.tensor, offset=eP[:].offset,
                    ap=[eP[:].ap[0], [8, 8], [1, 4], [0, 8]],
                )
                mc8_b = bass.AP(
                    tensor=mc8_s.tensor, offset=mc8_s[:].offset,
                    ap=[mc8_s[:].ap[0], [0, 8], [0, 4], [1, 8]],
                )
                nc.vector.tensor_mul(em[:], ei_b, mc8_b)

                # eirep[(b,j),(h,i)] per chunk via block-ones matmul
                erP = ps_e.tile([128, 8, 32], f32, tag="erP")
                for k in range(8):
                    nc.tensor.matmul(
                        erP[:, k, :],
                        bo_s[:],
                        em[:, k, :, :].rearrange("p h i -> p (h i)"),
                        start=True, stop=True,
                    )

                # e = eirep + ej (bcast over i), lrelu, exp
                ejs = work.tile([128, 8, 4], f32, tag="ejs")
                nc.vector.tensor_copy(ejs[:], eP[:, :, 4:8])
                e_all = work.tile([128, 8, 4, 8], f32, tag="e_all")
                ej_b = bass.AP(
                    tensor=ejs.tensor, offset=ejs[:].offset,
                    ap=[ejs[:].ap[0], [4, 8], [1, 4], [0, 8]],
                )
                nc.vector.tensor_add(
                    e_all[:],
                    erP[:].rearrange("p k (h i) -> p k h i", h=4),
                    ej_b,
                )
                el = work.tile([128, 8, 4, 8], f32, tag="el")
                nc.vector.scalar_tensor_tensor(
                    el[:], e_all[:], 0.2, e_all[:], AL.mult, AL.max
                )
                attn = work.tile([128, 8, 4, 8], f32, tag="attn")
                nc.scalar.activation(attn[:], el[:], AF.Exp)

                # block-diag attn and s-matmuls, in two 4-chunk halves
                g = bigwork.tile([128, 8, 4, 32], f32, tag="g")
                z = work.tile([128, 8, 4], f32, tag="z")
                for half in range(2):
                    sP = ps_s.tile([128, 4, 4, 64], f32, tag="sP")
                    for kk in range(4):
                        k = half * 4 + kk
                        bdk = bigwork.tile([128, 4, 16, 8], f32, tag="bdk")
                        attn_b = bass.AP(
                            tensor=attn.tensor, offset=attn[:].offset + k * 32,
                            ap=[attn[:].ap[0], [8, 4], [0, 16], [1, 8]],
                        )
                        mc16_b = bass.AP(
                            tensor=mc16_s.tensor, offset=mc16_s[:].offset,
                            ap=[mc16_s[:].ap[0], [0, 4], [1, 16], [0, 8]],
                        )
                        nc.vector.tensor_mul(bdk[:], attn_b, mc16_b)
                        for h in range(4):
                            nc.tensor.matmul(
                                sP[:, kk, h, 0:33],
                                bdk[:, h, :, :].rearrange("p b i -> p (b i)"),
                                hps[:, k, h, :],
                                start=True, stop=True,
                            )
                    # den -> rden
                    rden = work.tile([128, 4, 4], f32, tag="rden")
                    nc.vector.reciprocal(rden[:], sP[:, :, :, 32])
                    # normalize
                    s_n = bigwork.tile([128, 4, 4, 32], f32, tag="s_n")
                    rden_b = bass.AP(
                        tensor=rden.tensor, offset=rden[:].offset,
                        ap=[rden[:].ap[0], [4, 4], [1, 4], [0, 32]],
                    )
                    nc.vector.tensor_mul(s_n[:], sP[:, :, :, 0:32], rden_b)
                    # elu
                    exps = bigwork.tile([128, 4, 4, 32], f32, tag="exps")
                    nc.scalar.activation(exps[:], s_n[:], AF.Exp)
                    t1 = bigwork.tile([128, 4, 4, 32], f32, tag="t1")
                    nc.vector.tensor_scalar(t1[:], exps[:], -1.0, 0.0, AL.add, AL.min)
                    nc.vector.scalar_tensor_tensor(
                        g[:, half * 4:half * 4 + 4, :, :], s_n[:], 0.0, t1[:],
                        AL.max, AL.add,
                    )

                # z = sum_d wdvd * g
                # wdvd computed naturally, then replicated to (b_loc, i)
                # partitions per chunk via selector-constant matmuls.
                wdvdP = ps_small.tile([128, 128], f32, tag="psm")
                nc.tensor.matmul(wdvdP[:], st_hi[:], wda_hi[:], start=True, stop=False)
                nc.tensor.matmul(wdvdP[:], st_lo[:], wda_lo[:], start=False, stop=True)
                wdvds = work.tile([128, 128], f32, tag="wdvds")
                nc.vector.tensor_copy(wdvds[:], wdvdP[:])
                tmpz = bigwork.tile([128, 8, 4, 32], f32, tag="tmpz")
                for k in range(8):
                    wrP = ps_hp.tile([128, 128], f32, tag="hp")
                    nc.tensor.matmul(wrP[:], selk_s[:, k, :], wdvds[:], start=True, stop=True)
                    nc.vector.tensor_mul(
                        tmpz[:, k, :, :],
                        g[:, k, :, :],
                        wrP[:].rearrange("p (h d) -> p h d", h=4),
                    )
                nc.vector.tensor_reduce(z[:], tmpz[:], AX.X, AL.add)

                # aw1 = 0.25 * sum_h |z| - 1
                za = work.tile([128, 8, 4], f32, tag="za")
                nc.vector.scalar_tensor_tensor(za[:], z[:], -1.0, z[:], AL.mult, AL.max)
                aw = work.tile([128, 8], f32, tag="aw")
                nc.vector.tensor_reduce(aw[:], za[:], AX.X, AL.add)
                aw1 = work.tile([128, 8], f32, tag="aw1")
                nc.vector.tensor_scalar(aw1[:], aw[:], 0.25, -1.0, AL.mult, AL.add)

                # bridge (b_loc,i)-partitions -> natural rows
                awT2 = ps_small.tile([8, 128], f32, tag="psm")
                nc.tensor.matmul(awT2[:], aw1[:], ident_s[:], start=True, stop=True)
                awT2s = work.tile([8, 128], f32, tag="awT2s")
                nc.vector.tensor_copy(awT2s[:], awT2[:])
                awX = ps_small.tile([128, 128], f32, tag="psm")
                awT2_r = bass.AP(
                    tensor=awT2s.tensor, offset=awT2s[:].offset,
                    ap=[awT2s[:].ap[0], [1, 8], [8, 16]],
                )
                nc.tensor.matmul(awX[:], sel3_s[:], awT2_r, start=True, stop=True)
                tmpb = work.tile([128, 8, 16], f32, tag="tmpb")
                mc16b_b = bass.AP(
                    tensor=mc16b_s.tensor, offset=mc16b_s[:].offset,
                    ap=[mc16b_s[:].ap[0], [0, 8], [1, 16]],
                )
                awX_r = bass.AP(
                    tensor=awX.tensor, offset=awX[:].offset,
                    ap=[awX[:].ap[0], [16, 8], [1, 16]],
                )
                nc.vector.tensor_mul(tmpb[:], awX_r, mc16b_b)
                awN = work.tile([128, 8], f32, tag="awN")
                nc.vector.tensor_reduce(awN[:], tmpb[:], AX.X, AL.add)

                # final: out = sum_i advq * awN
                term = work.tile([128, 8], f32, tag="term")
                nc.vector.tensor_mul(term[:], advq[:], awN[:])
                adv = work.tile([128, 1], f32, tag="adv")
                nc.vector.tensor_reduce(adv[:], term[:], AX.X, AL.add)
                nc.sync.dma_start(out[c0:c0 + 128, :], adv[:])

                if debug and t == 0:
                    nc.sync.dma_start(dbg["d_advq"][:], advq[:])
                    nc.sync.dma_start(dbg["d_aw1"][:], aw1[:])
                    nc.sync.dma_start(dbg["d_awn"][:], awN[:])
                    nc.sync.dma_start(dbg["d_eall"][:],
                                      e_all[:].rearrange("p k h i -> p (k h i)"))
                    nc.sync.dma_start(dbg["d_hps"][:],
                                      hps[:].rearrange("p k h i -> p (k h i)"))
                    nc.sync.dma_start(dbg["d_z"][:],
                                      z[:].rearrange("p k h -> p (k h)"))
                    nc.sync.dma_start(dbg["d_g"][:],
                                      g[:].rearrange("p k h d -> p (k h d)"))

    return nc


def _host_inputs(agent_qs, states, max_q_i, hidden_states,
                 w1f, b1f, w2f, b2f, Wg, att_a, Wd, bd):
    """Build per-core input maps (list of dicts)."""
    f = np.float32
    S = np.asarray(states, f).reshape(B_TOTAL, STATE_DIM)
    q = np.asarray(agent_qs, f).reshape(B_TOTAL, N_AGENTS)
    mq = np.asarray(max_q_i, f).reshape(B_TOTAL, N_AGENTS)
    H = np.asarray(hidden_states, f).reshape(B_TOTAL, N_AGENTS, RNN_DIM)

    w1a = np.concatenate([np.asarray(w1f, f), np.asarray(b1f, f)[None, :]], 0)
    w2a = np.concatenate([np.asarray(w2f, f), np.asarray(b2f, f)[None, :]], 0)
    wda = np.concatenate([np.asarray(Wd, f), np.asarray(bd, f)[None, :]], 0)
    Wg = np.asarray(Wg, f)                      # [64, 128] cols (h,d)
    Wg_r = Wg.reshape(RNN_DIM, N_HEADS, GAT_DIM)
    a_i = np.asarray(att_a, f)[0, :, :GAT_DIM]   # [h, d]
    a_j = np.asarray(att_a, f)[0, :, GAT_DIM:]
    wae = np.zeros((RNN_DIM, 8), f)
    wae[:, 0:4] = np.einsum("fhd,hd->fh", Wg_r, a_i)
    wae[:, 4:8] = np.einsum("fhd,hd->fh", Wg_r, a_j)

    # constant masks
    p = np.arange(128)
    bo = (p[:, None] // 8 == p[None, :] // 8).astype(f)          # 8x8 ones blocks
    mc8 = (np.arange(8)[None, :] == (p % 8)[:, None]).astype(f)  # [128, 8]
    mc16 = (np.arange(16)[None, :] == (p // 8)[:, None]).astype(f)
    mc16b = (np.arange(16)[None, :] == (p % 16)[:, None]).astype(f)
    ident = np.eye(128, dtype=f)
    sel3 = (np.arange(8)[:, None] == (p // 16)[None, :]).astype(f)  # [8, 128]
    # selk[p, k, m] = 1 if p == 16k + m//8  (replicate row 16k+b over i)
    selk = np.zeros((128, 8, 128), f)
    for k in range(8):
        selk[:, k, :] = (p[:, None] == (16 * k + p[None, :] // 8)).astype(f)
    selk = selk.reshape(128, 8 * 128)

    ones_row = np.ones((1, B), f)
    in_maps = []
    for c in range(N_CORES):
        r0, r1 = c * B, (c + 1) * B
        st_c = np.concatenate([S[r0:r1].T, ones_row], 0)             # [169, B]
        ht_c = np.ascontiguousarray(
            H[r0:r1].reshape(B * N_AGENTS, RNN_DIM).T)               # [64, B*8]
        qm_c = np.concatenate([q[r0:r1], mq[r0:r1]], 1)              # [B, 16]
        in_maps.append({
            "st": np.ascontiguousarray(st_c), "ht": ht_c,
            "qm": np.ascontiguousarray(qm_c),
            "w1a": w1a, "w2a": w2a, "wg": Wg, "wae": wae, "wda": wda,
            "bo": bo, "mc8": mc8, "mc16": mc16, "mc16b": mc16b,
            "ident": ident, "sel3": sel3, "selk": selk,
        })
    return in_maps


def kernel(agent_qs, states, max_q_i, hidden_states,
           w1f, b1f, w2f, b2f, w1v, b1v, w2v, b2v,
           Wg, att_a, Wd, bd, **_unused):
    from concourse.bass_utils import run_bass_kernel_spmd

    if "nc" not in _CACHE:
        nc = _build_bass()
        nc.finalize()
        _CACHE["nc"] = nc
    nc = _CACHE["nc"]

    in_maps = _host_inputs(agent_qs, states, max_q_i, hidden_states,
                           w1f, b1f, w2f, b2f, Wg, att_a, Wd, bd)
    res = run_bass_kernel_spmd(nc, in_maps, list(range(N_CORES)))
    outs = [np.asarray(res.results[c]["out"]).reshape(B) for c in range(N_CORES)]
    full = np.concatenate(outs, 0).astype(np.float32)
    bs = np.asarray(agent_qs).shape[0]
    return full.reshape(bs, -1, 1)
